# revision 14
# baseline (speedup 1.0000x reference)
import numpy as np

# nn_Head: single-head causal attention.
# B=8, T=2048, E=1024, D=128. Data-parallel: one batch element per core.
# Per core: q/k/v projections (bf16 matmuls), causal softmax(q k^T / sqrt(D)) @ v.
#
# Layout trick: compute S^T = K @ q^T directly ([key, query]); then
# P^T = exp(S^T) is exactly the stationary (lhsT) operand needed by the
# P @ V matmul, so no PE transposes are needed. The softmax row-sum is
# obtained for free by appending a ones-column to V (N = D+1 = 129), and
# the max-subtraction is skipped (scores are O(1), exp is safe in fp32).
B, T, E, D = 8, 2048, 1024, 128
SCALE = 1.0 / np.sqrt(D)
NT = T // 128        # 16 query/key row tiles
NE = E // 128        # 8 contraction chunks
NC_CHUNK = T // 512  # 4 query chunks of 512


def _build():
    from concourse import bacc, bass, tile
    from concourse.bass import mybir

    f32 = mybir.dt.float32
    bf16 = mybir.dt.bfloat16
    nc = bacc.Bacc(None, target_bir_lowering=False)

    XT_d = nc.declare_dram_parameter("XT", [E, T], bf16, isOutput=False)
    Wq_d = nc.declare_dram_parameter("Wq", [E, D], bf16, isOutput=False)
    Wk_d = nc.declare_dram_parameter("Wk", [E, D], bf16, isOutput=False)
    Wv_d = nc.declare_dram_parameter("Wv", [E, D], bf16, isOutput=False)
    maskT_d = nc.declare_dram_parameter("maskT", [128, 128], f32, isOutput=False)
    out_d = nc.declare_dram_parameter("out", [T, D], bf16, isOutput=True)

    with tile.TileContext(nc) as tc:
        with (
            tc.tile_pool(name="persist", bufs=1) as pp,
            tc.tile_pool(name="pt", bufs=12) as ptp,
            tc.tile_pool(name="ostage", bufs=3) as wp,
            tc.tile_pool(name="rstage", bufs=3) as rp,
            tc.tile_pool(name="spsum", bufs=4, space=bass.MemorySpace.PSUM) as sp,
            tc.tile_pool(name="apsum", bufs=4, space=bass.MemorySpace.PSUM) as ap,
        ):
            XT = pp.tile([128, NE, T], bf16)      # X^T: [e, t]
            Wq = pp.tile([128, NE, D], bf16)
            Wk = pp.tile([128, NE, D], bf16)
            Wv = pp.tile([128, NE, D], bf16)
            qT = pp.tile([128, T], bf16)          # q^T [d, t]
            kT = pp.tile([128, T], bf16)          # k^T [d, t]
            v = pp.tile([128, NT, D + 1], bf16)   # v [t, d] row-tiled, col D = ones
            maskT = pp.tile([128, 128], f32)      # [k, q]: -1e30 where k > q

            # DMA order: the first 16 transfers (Wq + X^T column chunk 0) are
            # exactly what the first projection needs; they round-robin onto
            # all 16 DMA queues. X^T goes in [128, 512] pieces per chunk.
            Ws = (Wq, Wk, Wv)
            Wds = (Wq_d, Wk_d, Wv_d)
            for c in range(NC_CHUNK):
                if c < 3:
                    W, W_d = Ws[c], Wds[c]
                    for e in range(NE):
                        nc.gpsimd.dma_start(
                            W[:, e, :], W_d[e * 128:(e + 1) * 128, :])
                else:
                    nc.gpsimd.dma_start(maskT[:], maskT_d[:])
                for e in range(NE):
                    nc.gpsimd.dma_start(
                        XT[:, e, c * 512:(c + 1) * 512],
                        XT_d[e * 128:(e + 1) * 128, c * 512:(c + 1) * 512])

            nc.vector.memset(v[:, :, D:D + 1], 1.0)

            Exp = mybir.ActivationFunctionType.Exp
            Add = mybir.AluOpType.add

            def emit_S(qc, j):
                # S^T block [key tile j (128), query chunk qc (512)] -> P^T bf16
                live0 = max(0, j - 4 * qc) * 128
                q0 = qc * 512
                S = sp.tile([128, 512], f32, name="S")
                nc.tensor.matmul(
                    S[:, live0:512],
                    kT[:, j * 128:(j + 1) * 128],
                    qT[:, q0 + live0:q0 + 512],
                    start=True, stop=True)
                if j >= 4 * qc:  # diagonal block: causal mask
                    nc.vector.tensor_tensor(
                        S[:, live0:live0 + 128], S[:, live0:live0 + 128],
                        maskT[:], op=Add)
                Pt = ptp.tile([128, 512], bf16, name="Pt")
                nc.scalar.activation(
                    Pt[:, live0:512], S[:, live0:512], Exp, bias=0.0, scale=SCALE)
                return Pt

            # q/k projections: qT/kT [d, t] = W^T @ X^T, 512-wide t chunks
            for c in range(NC_CHUNK):
                for W, dst in ((Wq, qT), (Wk, kT)):
                    ps = sp.tile([128, 512], f32, name="S")
                    for e in range(NE):
                        nc.tensor.matmul(
                            ps[:], W[:, e, :], XT[:, e, c * 512:(c + 1) * 512],
                            start=(e == 0), stop=(e == NE - 1))
                    nc.vector.tensor_copy(dst[:, c * 512:(c + 1) * 512], ps[:])
                if c == 0:
                    # queries 0..511 attend only to keys 0..511: S^T for
                    # chunk 0 can run now, letting exp overlap the v phase.
                    pts0 = [emit_S(0, j) for j in range(4)]

            # v: [t, d] = X @ Wv, one 128-row tile at a time. Interleave the
            # S^T/exp emissions for query chunk 1 so the scalar engine's exp
            # work overlaps the PE's v matmuls instead of the attention PVs.
            pts1 = [None] * 8
            for t in range(NT):
                ps = ap.tile([128, 512], f32, name="acc")
                for e in range(NE):
                    nc.tensor.matmul(
                        ps[:, 0:D], XT[:, e, t * 128:(t + 1) * 128], Wv[:, e, :],
                        start=(e == 0), stop=(e == NE - 1))
                nc.vector.tensor_copy(v[:, t, 0:D], ps[:, 0:D])
                if t % 2 == 1 and t // 2 < 6:
                    pts1[t // 2] = emit_S(1, t // 2)

            # attention: per query chunk, accumulate P @ [V | 1] over key tiles
            for qc in range(NC_CHUNK):
                nj = 4 * qc + 4
                accs = [ap.tile([128, 512], f32, name="acc") for i in range(4)]
                if qc == 0:
                    pts = pts0
                elif qc == 1:
                    pts = pts1
                    pts[6] = emit_S(1, 6)
                    pts[7] = emit_S(1, 7)
                else:
                    pts = [None] * nj
                    for jj in range(3):
                        pts[jj] = emit_S(qc, jj)
                for j in range(nj):
                    if qc > 1 and j + 3 < nj:
                        pts[j + 3] = emit_S(qc, j + 3)
                    for tl in range(4):
                        tg = 4 * qc + tl
                        if tg < j:
                            continue
                        nc.tensor.matmul(
                            accs[tl][:, 0:D + 1],
                            pts[j][:, tl * 128:(tl + 1) * 128],
                            v[:, j, 0:D + 1],
                            start=(j == 0), stop=(j == tg))
                        if j == tg:
                            rcp = rp.tile([128, 1], f32)
                            nc.vector.reciprocal(rcp[:], accs[tl][:, D:D + 1])
                            o = wp.tile([128, D], bf16)
                            nc.vector.tensor_scalar_mul(
                                o[:], accs[tl][:, 0:D], rcp[:])
                            nc.gpsimd.dma_start(
                                out_d[tg * 128:(tg + 1) * 128, :], o[:])

    nc.compile()
    return nc


_NC = None
LAST_RESULT = None


def kernel(X, Wq, Wk, Wv):
    global _NC, LAST_RESULT
    import ml_dtypes
    from concourse.bass_utils import run_bass_kernel_spmd

    if _NC is None:
        _NC = _build()
    bf = ml_dtypes.bfloat16
    Xb = np.asarray(X, np.float32).astype(bf)            # [B, T, E]
    XTb = np.ascontiguousarray(Xb.transpose(0, 2, 1))    # [B, E, T]
    maskT = np.tril(np.full((128, 128), -1e30, np.float32), -1)
    base = {
        "Wq": np.ascontiguousarray(np.asarray(Wq, np.float32).astype(bf)),
        "Wk": np.ascontiguousarray(np.asarray(Wk, np.float32).astype(bf)),
        "Wv": np.ascontiguousarray(np.asarray(Wv, np.float32).astype(bf)),
        "maskT": maskT,
    }
    in_maps = [dict(base, XT=XTb[b]) for b in range(B)]
    res = run_bass_kernel_spmd(_NC, in_maps, core_ids=list(range(B)))
    LAST_RESULT = res
    outs = []
    for r in res.results:
        o = np.asarray(r["out"] if isinstance(r, dict) else r)
        outs.append(o.astype(np.float32))
    return np.stack(outs, 0).reshape(B, T, D)


# revision 15
# speedup vs baseline: 1.2724x; 1.2724x over previous
import numpy as np

# nn_Head: single-head causal attention.
# B=8, T=2048, E=1024, D=128. Data-parallel: one batch element per core.
# Per core: q/k/v projections (bf16 matmuls), causal softmax(q k^T / sqrt(D)) @ v.
#
# Layout trick: compute S^T = K @ q^T directly ([key, query]); then
# P^T = exp(S^T) is exactly the stationary (lhsT) operand needed by the
# P @ V matmul, so no PE transposes are needed. The softmax row-sum is
# obtained for free by appending a ones-column to V (N = D+1 = 129), and
# the max-subtraction is skipped (scores are O(1), exp is safe in fp32).
B, T, E, D = 8, 2048, 1024, 128
SCALE = 1.0 / np.sqrt(D)
NT = T // 128        # 16 query/key row tiles
NE = E // 128        # 8 contraction chunks
NC_CHUNK = T // 512  # 4 query chunks of 512


def _build():
    from concourse import bacc, bass, tile
    from concourse.bass import mybir

    f32 = mybir.dt.float32
    bf16 = mybir.dt.bfloat16
    nc = bacc.Bacc(None, target_bir_lowering=False)

    XT_d = nc.declare_dram_parameter("XT", [E, T], bf16, isOutput=False)
    Wq_d = nc.declare_dram_parameter("Wq", [E, D], bf16, isOutput=False)
    Wk_d = nc.declare_dram_parameter("Wk", [E, D], bf16, isOutput=False)
    Wv_d = nc.declare_dram_parameter("Wv", [E, D], bf16, isOutput=False)
    maskT_d = nc.declare_dram_parameter("maskT", [128, 128], f32, isOutput=False)
    out_d = nc.declare_dram_parameter("out", [T, D], bf16, isOutput=True)

    with tile.TileContext(nc) as tc:
        with (
            tc.tile_pool(name="persist", bufs=1) as pp,
            tc.tile_pool(name="pt", bufs=12) as ptp,
            tc.tile_pool(name="ostage", bufs=3) as wp,
            tc.tile_pool(name="rstage", bufs=3) as rp,
            tc.tile_pool(name="spsum", bufs=4, space=bass.MemorySpace.PSUM) as sp,
            tc.tile_pool(name="apsum", bufs=4, space=bass.MemorySpace.PSUM) as ap,
        ):
            XT = pp.tile([128, NE, T], bf16)      # X^T: [e, t]
            Wq = pp.tile([128, NE, D], bf16)
            Wk = pp.tile([128, NE, D], bf16)
            Wv = pp.tile([128, NE, D], bf16)
            qT = pp.tile([128, T], bf16)          # q^T [d, t]
            kT = pp.tile([128, T], bf16)          # k^T [d, t]
            v = pp.tile([128, NT, D + 1], bf16)   # v [t, d] row-tiled, col D = ones
            maskT = pp.tile([128, 128], f32)      # [k, q]: -1e30 where k > q

            # DMA descriptor generation costs ~0.7us serially per dma_start
            # on the issuing engine, so keep the count tiny and split issue
            # across two engines. Weights+mask (4 descs) go on sync/SP via
            # 3D rearranged views; X^T goes on gpsimd as 4 column-chunk
            # pieces so the first projection chunk unblocks early.
            nc.sync.dma_start(Wq[:], Wq_d[:].rearrange("(e p) d -> p e d", p=128))
            nc.sync.dma_start(maskT[:], maskT_d[:])
            nc.sync.dma_start(Wk[:], Wk_d[:].rearrange("(e p) d -> p e d", p=128))
            nc.sync.dma_start(Wv[:], Wv_d[:].rearrange("(e p) d -> p e d", p=128))
            XT_src = XT_d[:].rearrange("(e p) t -> p e t", p=128)
            for c in range(NC_CHUNK):
                nc.gpsimd.dma_start(
                    XT[:, :, c * 512:(c + 1) * 512],
                    XT_src[:, :, c * 512:(c + 1) * 512])

            nc.vector.memset(v[:, :, D:D + 1], 1.0)

            Exp = mybir.ActivationFunctionType.Exp
            Add = mybir.AluOpType.add

            def emit_S(qc, j):
                # S^T block [key tile j (128), query chunk qc (512)] -> P^T bf16
                live0 = max(0, j - 4 * qc) * 128
                q0 = qc * 512
                S = sp.tile([128, 512], f32, name="S")
                nc.tensor.matmul(
                    S[:, live0:512],
                    kT[:, j * 128:(j + 1) * 128],
                    qT[:, q0 + live0:q0 + 512],
                    start=True, stop=True)
                if j >= 4 * qc:  # diagonal block: causal mask
                    nc.vector.tensor_tensor(
                        S[:, live0:live0 + 128], S[:, live0:live0 + 128],
                        maskT[:], op=Add)
                Pt = ptp.tile([128, 512], bf16, name="Pt")
                nc.scalar.activation(
                    Pt[:, live0:512], S[:, live0:512], Exp, bias=0.0, scale=SCALE)
                return Pt

            # q/k projections: qT/kT [d, t] = W^T @ X^T, 512-wide t chunks
            for c in range(NC_CHUNK):
                for W, dst in ((Wq, qT), (Wk, kT)):
                    ps = sp.tile([128, 512], f32, name="S")
                    for e in range(NE):
                        nc.tensor.matmul(
                            ps[:], W[:, e, :], XT[:, e, c * 512:(c + 1) * 512],
                            start=(e == 0), stop=(e == NE - 1))
                    nc.vector.tensor_copy(dst[:, c * 512:(c + 1) * 512], ps[:])
                if c == 0:
                    # queries 0..511 attend only to keys 0..511: S^T for
                    # chunk 0 can run now, letting exp overlap the v phase.
                    pts0 = [emit_S(0, j) for j in range(4)]

            # v: [t, d] = X @ Wv, one 128-row tile at a time. Interleave the
            # S^T/exp emissions for query chunk 1 so the scalar engine's exp
            # work overlaps the PE's v matmuls instead of the attention PVs.
            pts1 = [None] * 8
            for t in range(NT):
                ps = ap.tile([128, 512], f32, name="acc")
                for e in range(NE):
                    nc.tensor.matmul(
                        ps[:, 0:D], XT[:, e, t * 128:(t + 1) * 128], Wv[:, e, :],
                        start=(e == 0), stop=(e == NE - 1))
                nc.vector.tensor_copy(v[:, t, 0:D], ps[:, 0:D])
                if t % 2 == 1 and t // 2 < 6:
                    pts1[t // 2] = emit_S(1, t // 2)

            # attention: per query chunk, accumulate P @ [V | 1] over key tiles
            for qc in range(NC_CHUNK):
                nj = 4 * qc + 4
                accs = [ap.tile([128, 512], f32, name="acc") for i in range(4)]
                if qc == 0:
                    pts = pts0
                elif qc == 1:
                    pts = pts1
                    pts[6] = emit_S(1, 6)
                    pts[7] = emit_S(1, 7)
                else:
                    pts = [None] * nj
                    for jj in range(3):
                        pts[jj] = emit_S(qc, jj)
                for j in range(nj):
                    if qc > 1 and j + 3 < nj:
                        pts[j + 3] = emit_S(qc, j + 3)
                    for tl in range(4):
                        tg = 4 * qc + tl
                        if tg < j:
                            continue
                        nc.tensor.matmul(
                            accs[tl][:, 0:D + 1],
                            pts[j][:, tl * 128:(tl + 1) * 128],
                            v[:, j, 0:D + 1],
                            start=(j == 0), stop=(j == tg))
                        if j == tg:
                            rcp = rp.tile([128, 1], f32)
                            nc.vector.reciprocal(rcp[:], accs[tl][:, D:D + 1])
                            o = wp.tile([128, D], bf16)
                            nc.vector.tensor_scalar_mul(
                                o[:], accs[tl][:, 0:D], rcp[:])
                            nc.gpsimd.dma_start(
                                out_d[tg * 128:(tg + 1) * 128, :], o[:])

    nc.compile()
    return nc


_NC = None
LAST_RESULT = None


def kernel(X, Wq, Wk, Wv):
    global _NC, LAST_RESULT
    import ml_dtypes
    from concourse.bass_utils import run_bass_kernel_spmd

    if _NC is None:
        _NC = _build()
    bf = ml_dtypes.bfloat16
    Xb = np.asarray(X, np.float32).astype(bf)            # [B, T, E]
    XTb = np.ascontiguousarray(Xb.transpose(0, 2, 1))    # [B, E, T]
    maskT = np.tril(np.full((128, 128), -1e30, np.float32), -1)
    base = {
        "Wq": np.ascontiguousarray(np.asarray(Wq, np.float32).astype(bf)),
        "Wk": np.ascontiguousarray(np.asarray(Wk, np.float32).astype(bf)),
        "Wv": np.ascontiguousarray(np.asarray(Wv, np.float32).astype(bf)),
        "maskT": maskT,
    }
    in_maps = [dict(base, XT=XTb[b]) for b in range(B)]
    res = run_bass_kernel_spmd(_NC, in_maps, core_ids=list(range(B)))
    LAST_RESULT = res
    outs = []
    for r in res.results:
        o = np.asarray(r["out"] if isinstance(r, dict) else r)
        outs.append(o.astype(np.float32))
    return np.stack(outs, 0).reshape(B, T, D)


# revision 17
# speedup vs baseline: 1.6047x; 1.2612x over previous
import numpy as np

# nn_Head: single-head causal attention.
# B=8, T=2048, E=1024, D=128. Data-parallel: one batch element per core.
# Per core: q/k/v projections (bf16 matmuls), causal softmax(q k^T / sqrt(D)) @ v.
#
# Layout trick: compute S^T = K @ q^T directly ([key, query]); then
# P^T = exp(S^T) is exactly the stationary (lhsT) operand needed by the
# P @ V matmul, so no PE transposes are needed. The softmax row-sum is
# obtained for free by appending a ones-column to V (N = D+1 = 129), and
# the max-subtraction is skipped (scores are O(1), exp is safe in fp32).
#
# Scheduling notes (from perfetto traces):
# - DMA descriptor generation costs ~0.8us serially per dma_start on the
#   issuing engine; keep dma_start count minimal, split across engines.
# - Dependency tracking is per-tile for DMA writes: separate tiles per
#   independently-consumed DMA region (XT column chunks, v tile groups).
# - PSUM pool slots are reused round-robin per tag; giving S^T tiles a
#   dedicated tag keeps projections from serializing behind exp.
B, T, E, D = 8, 2048, 1024, 128
SCALE = 1.0 / np.sqrt(D)
NT = T // 128        # 16 query/key row tiles
NE = E // 128        # 8 contraction chunks
NCH = T // 512       # 4 query chunks of 512

_BASE = [0, 4, 12, 24]  # first S-tile index of each query chunk


def _build():
    from concourse import bacc, bass, tile
    from concourse.bass import mybir

    f32 = mybir.dt.float32
    bf16 = mybir.dt.bfloat16
    nc = bacc.Bacc(None, target_bir_lowering=False)

    # Host supplies X^T pre-rearranged to [128, NE, T] (partition-major).
    XT_d = nc.declare_dram_parameter("XTr", [128, NE * T], bf16, isOutput=False)
    Wq_d = nc.declare_dram_parameter("Wq", [E, D], bf16, isOutput=False)
    Wk_d = nc.declare_dram_parameter("Wk", [E, D], bf16, isOutput=False)
    Wv_d = nc.declare_dram_parameter("Wv", [E, D], bf16, isOutput=False)
    maskT_d = nc.declare_dram_parameter("maskT", [128, 128], f32, isOutput=False)
    out_d = nc.declare_dram_parameter("out", [T, D], bf16, isOutput=True)

    with tile.TileContext(nc) as tc:
        with (
            tc.tile_pool(name="persist", bufs=1) as pp,
            tc.tile_pool(name="pt", bufs=16) as ptp,
            tc.tile_pool(name="ostage", bufs=4) as wp,
            tc.tile_pool(name="rstage", bufs=4) as rp,
            tc.tile_pool(name="spsum", bufs=4, space=bass.MemorySpace.PSUM) as sp,
            tc.tile_pool(name="apsum", bufs=4, space=bass.MemorySpace.PSUM) as ap,
        ):
            XTc = [pp.tile([128, NE, 512], bf16, name=f"XTc{c}")
                   for c in range(NCH)]
            Wq = pp.tile([128, NE, D], bf16)
            Wk = pp.tile([128, NE, D], bf16)
            Wv = pp.tile([128, NE, D], bf16)
            qT = pp.tile([128, T], bf16)          # q^T [d, t]
            kT = pp.tile([128, T], bf16)          # k^T [d, t]
            # v [t, d] in 4 tile groups of 4 row-tiles; col D = ones
            vt = [pp.tile([128, 4, D + 1], bf16, name=f"vt{g}")
                  for g in range(NCH)]
            maskT = pp.tile([128, 128], f32)      # [k, q]: -1e30 where k > q

            # Loads: weights+mask on sync/SP (4 descs), X^T column chunks
            # on gpsimd (4 descs).
            nc.sync.dma_start(Wq[:], Wq_d[:].rearrange("(e p) d -> p e d", p=128))
            nc.sync.dma_start(maskT[:], maskT_d[:])
            nc.sync.dma_start(Wk[:], Wk_d[:].rearrange("(e p) d -> p e d", p=128))
            nc.sync.dma_start(Wv[:], Wv_d[:].rearrange("(e p) d -> p e d", p=128))
            XT_src = XT_d[:].rearrange("p (e t) -> p e t", e=NE)
            for c in range(NCH):
                nc.gpsimd.dma_start(
                    XTc[c][:], XT_src[:, :, c * 512:(c + 1) * 512])
            for g in range(NCH):
                nc.vector.memset(vt[g][:, :, D:D + 1], 1.0)

            Exp = mybir.ActivationFunctionType.Exp
            Add = mybir.AluOpType.add

            pts = {}

            def emit_S(qc, j):
                # S^T block [key tile j (128), query chunk qc (512)] -> P^T
                live0 = max(0, j - 4 * qc) * 128
                q0 = qc * 512
                S = sp.tile([128, 512], f32, name="S")
                nc.tensor.matmul(
                    S[:, live0:512],
                    kT[:, j * 128:(j + 1) * 128],
                    qT[:, q0 + live0:q0 + 512],
                    start=True, stop=True)
                if j >= 4 * qc:  # diagonal block: causal mask
                    nc.vector.tensor_tensor(
                        S[:, live0:live0 + 128], S[:, live0:live0 + 128],
                        maskT[:], op=Add)
                Pt = ptp.tile([128, 512], bf16, name="Pt")
                nc.scalar.activation(
                    Pt[:, live0:512], S[:, live0:512], Exp, bias=0.0, scale=SCALE)
                pts[(qc, j)] = Pt

            cursor = [0]
            order = [(qc, j) for qc in range(NCH) for j in range(4 * qc + 4)]

            def pump(upto):
                while cursor[0] <= upto and cursor[0] < len(order):
                    qc_, j_ = order[cursor[0]]
                    emit_S(qc_, j_)
                    cursor[0] += 1

            # q/k projections: qT/kT [d, t] = W^T @ X^T, 512-wide t chunks.
            # Projection psum shares the "acc" tag (drained fast by casts),
            # never the "S" tag (drained by exp).
            for c in range(NCH):
                for W, dst in ((Wq, qT), (Wk, kT)):
                    ps = ap.tile([128, 512], f32, name="acc")
                    for e in range(NE):
                        nc.tensor.matmul(
                            ps[:], W[:, e, :], XTc[c][:, e, :],
                            start=(e == 0), stop=(e == NE - 1))
                    nc.vector.tensor_copy(dst[:, c * 512:(c + 1) * 512], ps[:])
                if c == 0:
                    pump(3)  # S^T for query chunk 0: keys 0..511 only

            # v tile groups interleaved with attention chunks: group g's
            # projections, then chunk g's PV accumulation (which needs only
            # v groups <= g), with S^T/exp emission pumped 3 tiles ahead.
            for g in range(NCH):
                for tl in range(4):
                    t = 4 * g + tl
                    ps = ap.tile([128, 512], f32, name="acc")
                    c, t0 = t // 4, (t % 4) * 128
                    for e in range(NE):
                        nc.tensor.matmul(
                            ps[:, 0:D], XTc[c][:, e, t0:t0 + 128], Wv[:, e, :],
                            start=(e == 0), stop=(e == NE - 1))
                    nc.vector.tensor_copy(vt[g][:, tl, 0:D], ps[:, 0:D])
                    if g < 3:
                        pump(_BASE[g + 1] + tl - 2)

                qc = g
                nj = 4 * qc + 4
                accs = [ap.tile([128, 512], f32, name="acc") for i in range(4)]
                for j in range(nj):
                    pump(_BASE[qc] + j + 3)
                    Pt = pts.pop((qc, j))
                    for tl in range(4):
                        tg = 4 * qc + tl
                        if tg < j:
                            continue
                        nc.tensor.matmul(
                            accs[tl][:, 0:D + 1],
                            Pt[:, tl * 128:(tl + 1) * 128],
                            vt[j // 4][:, j % 4, 0:D + 1],
                            start=(j == 0), stop=(j == tg))
                        if j == tg:
                            rcp = rp.tile([128, 1], f32)
                            nc.vector.reciprocal(rcp[:], accs[tl][:, D:D + 1])
                            o = wp.tile([128, D], bf16)
                            nc.vector.tensor_scalar_mul(
                                o[:], accs[tl][:, 0:D], rcp[:])
                            eng = nc.sync if (qc == 3 and tl % 2 == 1) else nc.gpsimd
                            eng.dma_start(
                                out_d[tg * 128:(tg + 1) * 128, :], o[:])

    nc.compile()
    return nc


_NC = None
LAST_RESULT = None


def kernel(X, Wq, Wk, Wv):
    global _NC, LAST_RESULT
    import ml_dtypes
    from concourse.bass_utils import run_bass_kernel_spmd

    if _NC is None:
        _NC = _build()
    bf = ml_dtypes.bfloat16
    Xb = np.asarray(X, np.float32).astype(bf)            # [B, T, E]
    # [B, E, T] -> partition-major [B, 128, NE*T]: XTr[b, p, e*T+t] = X[b, t, e*128+p]
    XTb = np.ascontiguousarray(
        Xb.transpose(0, 2, 1).reshape(B, NE, 128, T).transpose(0, 2, 1, 3)
    ).reshape(B, 128, NE * T)
    maskT = np.tril(np.full((128, 128), -1e30, np.float32), -1)
    base = {
        "Wq": np.ascontiguousarray(np.asarray(Wq, np.float32).astype(bf)),
        "Wk": np.ascontiguousarray(np.asarray(Wk, np.float32).astype(bf)),
        "Wv": np.ascontiguousarray(np.asarray(Wv, np.float32).astype(bf)),
        "maskT": maskT,
    }
    in_maps = [dict(base, XTr=XTb[b]) for b in range(B)]
    res = run_bass_kernel_spmd(_NC, in_maps, core_ids=list(range(B)))
    LAST_RESULT = res
    outs = []
    for r in res.results:
        o = np.asarray(r["out"] if isinstance(r, dict) else r)
        outs.append(o.astype(np.float32))
    return np.stack(outs, 0).reshape(B, T, D)


# revision 20
# speedup vs baseline: 1.6919x; 1.0543x over previous
import numpy as np

# nn_Head: single-head causal attention.
# B=8, T=2048, E=1024, D=128. Data-parallel: one batch element per core.
# Per core: q/k/v projections (bf16 matmuls), causal softmax(q k^T / sqrt(D)) @ v.
#
# Layout trick: compute S^T = K @ q^T directly ([key, query]); then
# P^T = exp(S^T) is exactly the stationary (lhsT) operand needed by the
# P @ V matmul, so no PE transposes are needed. The softmax row-sum is
# obtained for free by appending a ones-column to V (N = D+1 = 129), and
# the max-subtraction is skipped (scores are O(1), exp is safe in fp32).
#
# Scheduling notes (from perfetto traces):
# - DMA descriptor generation costs ~0.8us serially per dma_start on the
#   issuing engine; keep dma_start count minimal, split across engines.
# - Dependency tracking is per-tile for DMA writes: separate tiles per
#   independently-consumed DMA region (XT column chunks, v tile groups).
# - PSUM pool slots are reused round-robin per tag; giving S^T tiles a
#   dedicated tag keeps projections from serializing behind exp.
B, T, E, D = 8, 2048, 1024, 128
SCALE = 1.0 / np.sqrt(D)
NT = T // 128        # 16 query/key row tiles
NE = E // 128        # 8 contraction chunks
NCH = T // 512       # 4 query chunks of 512

_BASE = [0, 4, 12, 24]  # first S-tile index of each query chunk


def _build():
    from concourse import bacc, bass, tile
    from concourse.bass import mybir

    f32 = mybir.dt.float32
    bf16 = mybir.dt.bfloat16
    nc = bacc.Bacc(None, target_bir_lowering=False)

    # Host supplies X^T and weights pre-rearranged to partition-major
    # [128, NE, *] so DMA descriptors are contiguous runs per partition.
    XT_d = nc.declare_dram_parameter("XTr", [128, NE * T], bf16, isOutput=False)
    Wq_d = nc.declare_dram_parameter("Wqr", [128, NE * D], bf16, isOutput=False)
    Wk_d = nc.declare_dram_parameter("Wkr", [128, NE * D], bf16, isOutput=False)
    Wv_d = nc.declare_dram_parameter("Wvr", [128, NE * D], bf16, isOutput=False)
    maskT_d = nc.declare_dram_parameter("maskT", [128, 128], f32, isOutput=False)
    out_d = nc.declare_dram_parameter("out", [T, D], bf16, isOutput=True)

    with tile.TileContext(nc) as tc:
        with (
            tc.tile_pool(name="persist", bufs=1) as pp,
            tc.tile_pool(name="pt", bufs=16) as ptp,
            tc.tile_pool(name="ostage", bufs=4) as wp,
            tc.tile_pool(name="rstage", bufs=4) as rp,
            tc.tile_pool(name="spsum", bufs=4, space=bass.MemorySpace.PSUM) as sp,
            tc.tile_pool(name="apsum", bufs=4, space=bass.MemorySpace.PSUM) as ap,
        ):
            XTc = [pp.tile([128, NE, 512], bf16, name=f"XTc{c}")
                   for c in range(NCH)]
            Wq = pp.tile([128, NE, D], bf16)
            Wk = pp.tile([128, NE, D], bf16)
            Wv = pp.tile([128, NE, D], bf16)
            qT = pp.tile([128, T], bf16)          # q^T [d, t]
            kT = pp.tile([128, T], bf16)          # k^T [d, t]
            # v [t, d] in 4 tile groups of 4 row-tiles; col D = ones
            vt = [pp.tile([128, 4, D + 1], bf16, name=f"vt{g}")
                  for g in range(NCH)]
            maskT = pp.tile([128, 128], f32)      # [k, q]: -1e30 where k > q

            # Loads: weights+mask on sync/SP (4 descs), X^T column chunks
            # on gpsimd (4 descs).
            nc.sync.dma_start(Wq[:], Wq_d[:].rearrange("p (e d) -> p e d", e=NE))
            nc.sync.dma_start(maskT[:], maskT_d[:])
            nc.sync.dma_start(Wk[:], Wk_d[:].rearrange("p (e d) -> p e d", e=NE))
            nc.sync.dma_start(Wv[:], Wv_d[:].rearrange("p (e d) -> p e d", e=NE))
            XT_src = XT_d[:].rearrange("p (e t) -> p e t", e=NE)
            for c in range(NCH):
                nc.gpsimd.dma_start(
                    XTc[c][:], XT_src[:, :, c * 512:(c + 1) * 512])
            for g in range(NCH):
                nc.vector.memset(vt[g][:, :, D:D + 1], 1.0)

            Exp = mybir.ActivationFunctionType.Exp
            Add = mybir.AluOpType.add

            pts = {}

            def emit_S(qc, j):
                # S^T block [key tile j (128), query chunk qc (512)] -> P^T
                live0 = max(0, j - 4 * qc) * 128
                q0 = qc * 512
                S = sp.tile([128, 512], f32, name="S")
                nc.tensor.matmul(
                    S[:, live0:512],
                    kT[:, j * 128:(j + 1) * 128],
                    qT[:, q0 + live0:q0 + 512],
                    start=True, stop=True)
                if j >= 4 * qc:  # diagonal block: causal mask
                    nc.vector.tensor_tensor(
                        S[:, live0:live0 + 128], S[:, live0:live0 + 128],
                        maskT[:], op=Add)
                Pt = ptp.tile([128, 512], bf16, name="Pt")
                nc.scalar.activation(
                    Pt[:, live0:512], S[:, live0:512], Exp, bias=0.0, scale=SCALE)
                pts[(qc, j)] = Pt

            cursor = [0]
            order = [(qc, j) for qc in range(NCH) for j in range(4 * qc + 4)]

            def pump(upto):
                while cursor[0] <= upto and cursor[0] < len(order):
                    qc_, j_ = order[cursor[0]]
                    emit_S(qc_, j_)
                    cursor[0] += 1

            # q/k projections: qT/kT [d, t] = W^T @ X^T, 512-wide t chunks.
            # Projection psum shares the "acc" tag (drained fast by casts),
            # never the "S" tag (drained by exp).
            for c in range(NCH):
                for W, dst in ((Wq, qT), (Wk, kT)):
                    ps = ap.tile([128, 512], f32, name="acc")
                    for e in range(NE):
                        nc.tensor.matmul(
                            ps[:], W[:, e, :], XTc[c][:, e, :],
                            start=(e == 0), stop=(e == NE - 1))
                    nc.vector.tensor_copy(dst[:, c * 512:(c + 1) * 512], ps[:])
                if c == 0:
                    pump(3)  # S^T for query chunk 0: keys 0..511 only

            # v tile groups interleaved with attention chunks: group g's
            # projections, then chunk g's PV accumulation (which needs only
            # v groups <= g), with S^T/exp emission pumped 3 tiles ahead.
            for g in range(NCH):
                for tl in range(4):
                    t = 4 * g + tl
                    ps = ap.tile([128, 512], f32, name="acc")
                    c, t0 = t // 4, (t % 4) * 128
                    for e in range(NE):
                        nc.tensor.matmul(
                            ps[:, 0:D], XTc[c][:, e, t0:t0 + 128], Wv[:, e, :],
                            start=(e == 0), stop=(e == NE - 1))
                    nc.vector.tensor_copy(vt[g][:, tl, 0:D], ps[:, 0:D])
                    if g < 3:
                        pump(_BASE[g + 1] + tl - 2)

                qc = g
                nj = 4 * qc + 4
                accs = [ap.tile([128, 512], f32, name="acc") for i in range(4)]
                for j in range(nj):
                    pump(_BASE[qc] + j + 3)
                    Pt = pts.pop((qc, j))
                    for tl in range(4):
                        tg = 4 * qc + tl
                        if tg < j:
                            continue
                        nc.tensor.matmul(
                            accs[tl][:, 0:D + 1],
                            Pt[:, tl * 128:(tl + 1) * 128],
                            vt[j // 4][:, j % 4, 0:D + 1],
                            start=(j == 0), stop=(j == tg))
                        if j == tg:
                            rcp = rp.tile([128, 1], f32)
                            nc.vector.reciprocal(rcp[:], accs[tl][:, D:D + 1])
                            o = wp.tile([128, D], bf16)
                            nc.vector.tensor_scalar_mul(
                                o[:], accs[tl][:, 0:D], rcp[:])
                            eng = nc.sync if (qc == 3 and tl % 2 == 1) else nc.gpsimd
                            eng.dma_start(
                                out_d[tg * 128:(tg + 1) * 128, :], o[:])

    nc.compile()
    return nc


_NC = None
LAST_RESULT = None


def kernel(X, Wq, Wk, Wv):
    global _NC, LAST_RESULT
    import ml_dtypes
    from concourse.bass_utils import run_bass_kernel_spmd

    if _NC is None:
        _NC = _build()
    bf = ml_dtypes.bfloat16
    Xb = np.asarray(X, np.float32).astype(bf)            # [B, T, E]
    # [B, E, T] -> partition-major [B, 128, NE*T]: XTr[b, p, e*T+t] = X[b, t, e*128+p]
    XTb = np.ascontiguousarray(
        Xb.transpose(0, 2, 1).reshape(B, NE, 128, T).transpose(0, 2, 1, 3)
    ).reshape(B, 128, NE * T)
    maskT = np.tril(np.full((128, 128), -1e30, np.float32), -1)

    def _wr(W):
        # [E, D] -> partition-major [128, NE*D]: Wr[p, e*D+d] = W[e*128+p, d]
        Wb = np.asarray(W, np.float32).astype(bf)
        return np.ascontiguousarray(
            Wb.reshape(NE, 128, D).transpose(1, 0, 2)).reshape(128, NE * D)

    base = {
        "Wqr": _wr(Wq), "Wkr": _wr(Wk), "Wvr": _wr(Wv), "maskT": maskT,
    }
    in_maps = [dict(base, XTr=XTb[b]) for b in range(B)]
    res = run_bass_kernel_spmd(_NC, in_maps, core_ids=list(range(B)))
    LAST_RESULT = res
    outs = []
    for r in res.results:
        o = np.asarray(r["out"] if isinstance(r, dict) else r)
        outs.append(o.astype(np.float32))
    return np.stack(outs, 0).reshape(B, T, D)


# revision 25
# speedup vs baseline: 1.7366x; 1.0264x over previous
import numpy as np

# nn_Head: single-head causal attention.
# B=8, T=2048, E=1024, D=128. Data-parallel: one batch element per core.
# Per core: q/k/v projections (bf16 matmuls), causal softmax(q k^T / sqrt(D)) @ v.
#
# Layout trick: compute S^T = K @ q^T directly ([key, query]); then
# P^T = exp(S^T) is exactly the stationary (lhsT) operand needed by the
# P @ V matmul, so no PE transposes are needed. The softmax row-sum is
# obtained for free by appending a ones-column to V (N = D+1 = 129), and
# the max-subtraction is skipped (scores are O(1), exp is safe in fp32).
#
# Scheduling notes (from perfetto traces):
# - DMA descriptor generation costs ~0.8us serially per dma_start on the
#   issuing engine; keep dma_start count minimal, split across engines.
# - Dependency tracking is per-tile for DMA writes: separate tiles per
#   independently-consumed DMA region (XT column chunks, v tile groups).
# - PSUM pool slots are reused round-robin per tag; giving S^T tiles a
#   dedicated tag keeps projections from serializing behind exp.
B, T, E, D = 8, 2048, 1024, 128
SCALE = 1.0 / np.sqrt(D)
NT = T // 128        # 16 query/key row tiles
NE = E // 128        # 8 contraction chunks
NCH = T // 512       # 4 query chunks of 512

_BASE = [0, 4, 12, 24]  # first S-tile index of each query chunk


def _build():
    from concourse import bacc, bass, tile
    from concourse.bass import mybir

    f32 = mybir.dt.float32
    bf16 = mybir.dt.bfloat16
    nc = bacc.Bacc(None, target_bir_lowering=False)

    # Host supplies X^T and weights pre-rearranged to partition-major
    # [128, NE, *] so DMA descriptors are contiguous runs per partition.
    XT_d = nc.declare_dram_parameter("XTr", [128, NE * T], bf16, isOutput=False)
    Wq_d = nc.declare_dram_parameter("Wqr", [128, NE * D], bf16, isOutput=False)
    Wk_d = nc.declare_dram_parameter("Wkr", [128, NE * D], bf16, isOutput=False)
    Wv_d = nc.declare_dram_parameter("Wvr", [128, NE * D], bf16, isOutput=False)
    maskT_d = nc.declare_dram_parameter("maskT", [128, 128], f32, isOutput=False)
    out_d = nc.declare_dram_parameter("out", [T, D], bf16, isOutput=True)

    with tile.TileContext(nc) as tc:
        with (
            tc.tile_pool(name="persist", bufs=1) as pp,
            tc.tile_pool(name="pt", bufs=16) as ptp,
            tc.tile_pool(name="ostage", bufs=4) as wp,
            tc.tile_pool(name="rstage", bufs=4) as rp,
            tc.tile_pool(name="spsum", bufs=4, space=bass.MemorySpace.PSUM) as sp,
            tc.tile_pool(name="apsum", bufs=4, space=bass.MemorySpace.PSUM) as ap,
        ):
            # X^T per (column chunk, e-half): [128, 4, 512] tiles, each
            # loaded by one DMA whose source is 4KB-contiguous per partition.
            XTh = [[pp.tile([128, 4, 512], bf16, name=f"XTh{c}_{h}")
                    for h in range(2)] for c in range(NCH)]

            def xchunk(c, e):
                return XTh[c][e // 4][:, e % 4, :]
            Wq = pp.tile([128, NE, D], bf16)
            Wk = pp.tile([128, NE, D], bf16)
            Wv = pp.tile([128, NE, D], bf16)
            qT = pp.tile([128, T], bf16)          # q^T [d, t]
            kT = pp.tile([128, T], bf16)          # k^T [d, t]
            # v [t, d] in 4 tile groups of 4 row-tiles; col D = ones
            vt = [pp.tile([128, 4, D + 1], bf16, name=f"vt{g}")
                  for g in range(NCH)]
            maskT = pp.tile([128, 128], f32)      # [k, q]: -1e30 where k > q

            # Loads: weights+mask on sync/SP (4 descs), X^T column chunks
            # on gpsimd (4 descs).
            def xsrc(c, h):
                lo = (c * 2 + h) * 2048
                return XT_d[:, lo:lo + 2048].rearrange("p (e t) -> p e t", e=4)

            nc.gpsimd.dma_start(XTh[0][0][:], xsrc(0, 0))
            nc.sync.dma_start(Wq[:], Wq_d[:].rearrange("p (e d) -> p e d", e=NE))
            nc.sync.dma_start(XTh[0][1][:], xsrc(0, 1))
            nc.sync.dma_start(maskT[:], maskT_d[:])
            nc.sync.dma_start(Wk[:], Wk_d[:].rearrange("p (e d) -> p e d", e=NE))
            nc.sync.dma_start(Wv[:], Wv_d[:].rearrange("p (e d) -> p e d", e=NE))
            for c in range(1, NCH):
                nc.gpsimd.dma_start(XTh[c][0][:], xsrc(c, 0))
                nc.sync.dma_start(XTh[c][1][:], xsrc(c, 1))
            for g in range(NCH):
                nc.vector.memset(vt[g][:, :, D:D + 1], 1.0)

            Exp = mybir.ActivationFunctionType.Exp
            Add = mybir.AluOpType.add

            pts = {}

            def emit_S(qc, j):
                # S^T block [key tile j (128), query chunk qc (512)] -> P^T
                live0 = max(0, j - 4 * qc) * 128
                q0 = qc * 512
                S = sp.tile([128, 512], f32, name="S")
                nc.tensor.matmul(
                    S[:, live0:512],
                    kT[:, j * 128:(j + 1) * 128],
                    qT[:, q0 + live0:q0 + 512],
                    start=True, stop=True)
                if j >= 4 * qc:  # diagonal block: causal mask
                    nc.vector.tensor_tensor(
                        S[:, live0:live0 + 128], S[:, live0:live0 + 128],
                        maskT[:], op=Add)
                Pt = ptp.tile([128, 512], bf16, name="Pt")
                nc.scalar.activation(
                    Pt[:, live0:512], S[:, live0:512], Exp, bias=0.0, scale=SCALE)
                pts[(qc, j)] = Pt

            cursor = [0]
            order = [(qc, j) for qc in range(NCH) for j in range(4 * qc + 4)]

            def pump(upto):
                while cursor[0] <= upto and cursor[0] < len(order):
                    qc_, j_ = order[cursor[0]]
                    emit_S(qc_, j_)
                    cursor[0] += 1

            # q/k projections: qT/kT [d, t] = W^T @ X^T, 512-wide t chunks.
            # Projection psum shares the "acc" tag (drained fast by casts),
            # never the "S" tag (drained by exp).
            for c in range(NCH):
                for W, dst in ((Wq, qT), (Wk, kT)):
                    ps = ap.tile([128, 512], f32, name="acc")
                    for e in range(NE):
                        nc.tensor.matmul(
                            ps[:], W[:, e, :], xchunk(c, e),
                            start=(e == 0), stop=(e == NE - 1))
                    nc.vector.tensor_copy(dst[:, c * 512:(c + 1) * 512], ps[:])
                if c == 0:
                    pump(3)  # S^T for query chunk 0: keys 0..511 only

            # v tile groups interleaved with attention chunks: group g's
            # projections, then chunk g's PV accumulation (which needs only
            # v groups <= g), with S^T/exp emission pumped 3 tiles ahead.
            for g in range(NCH):
                for tl in range(4):
                    t = 4 * g + tl
                    ps = ap.tile([128, 512], f32, name="acc")
                    c, t0 = t // 4, (t % 4) * 128
                    for e in range(NE):
                        nc.tensor.matmul(
                            ps[:, 0:D], xchunk(c, e)[:, t0:t0 + 128], Wv[:, e, :],
                            start=(e == 0), stop=(e == NE - 1))
                    nc.vector.tensor_copy(vt[g][:, tl, 0:D], ps[:, 0:D])
                    if g < 3:
                        pump(_BASE[g + 1] + tl - 2)

                qc = g
                nj = 4 * qc + 4
                accs = [ap.tile([128, 512], f32, name="acc") for i in range(4)]
                for j in range(nj):
                    pump(_BASE[qc] + j + 3)
                    Pt = pts.pop((qc, j))
                    for tl in range(4):
                        tg = 4 * qc + tl
                        if tg < j:
                            continue
                        nc.tensor.matmul(
                            accs[tl][:, 0:D + 1],
                            Pt[:, tl * 128:(tl + 1) * 128],
                            vt[j // 4][:, j % 4, 0:D + 1],
                            start=(j == 0), stop=(j == tg))
                        if j == tg:
                            rcp = rp.tile([128, 1], f32)
                            nc.vector.reciprocal(rcp[:], accs[tl][:, D:D + 1])
                            o = wp.tile([128, D], bf16)
                            nc.vector.tensor_scalar_mul(
                                o[:], accs[tl][:, 0:D], rcp[:])
                            eng = nc.sync if (qc == 3 and tl % 2 == 1) else nc.gpsimd
                            eng.dma_start(
                                out_d[tg * 128:(tg + 1) * 128, :], o[:])

    nc.compile()
    return nc


_NC = None
LAST_RESULT = None


def kernel(X, Wq, Wk, Wv):
    global _NC, LAST_RESULT
    import ml_dtypes
    from concourse.bass_utils import run_bass_kernel_spmd

    if _NC is None:
        _NC = _build()
    bf = ml_dtypes.bfloat16
    Xb = np.asarray(X, np.float32).astype(bf)            # [B, T, E]
    # Partition-major, chunk-contiguous: XTr[b, p, ((c*2+h)*4+e')*512+t'] =
    # X[b, c*512+t', (h*4+e')*128+p] — each (c,h) block is 4KB contiguous
    # per partition, so its DMA is one contiguous run per partition.
    XTb = np.ascontiguousarray(
        Xb.reshape(B, NCH, 512, 2, 4, 128).transpose(0, 5, 1, 3, 4, 2)
    ).reshape(B, 128, NE * T)
    maskT = np.tril(np.full((128, 128), -1e30, np.float32), -1)

    def _wr(W):
        # [E, D] -> partition-major [128, NE*D]: Wr[p, e*D+d] = W[e*128+p, d]
        Wb = np.asarray(W, np.float32).astype(bf)
        return np.ascontiguousarray(
            Wb.reshape(NE, 128, D).transpose(1, 0, 2)).reshape(128, NE * D)

    base = {
        "Wqr": _wr(Wq), "Wkr": _wr(Wk), "Wvr": _wr(Wv), "maskT": maskT,
    }
    in_maps = [dict(base, XTr=XTb[b]) for b in range(B)]
    res = run_bass_kernel_spmd(_NC, in_maps, core_ids=list(range(B)))
    LAST_RESULT = res
    outs = []
    for r in res.results:
        o = np.asarray(r["out"] if isinstance(r, dict) else r)
        outs.append(o.astype(np.float32))
    return np.stack(outs, 0).reshape(B, T, D)
